# revision 3
# baseline (speedup 1.0000x reference)
"""Trainium2 Bass kernel for nn_LoRAMoEModule (fused base linear + LoRA + MoE-LoRA).

Computes, for x:[B,S,D], W:[Dout,D], b:[Dout], LoRA base (rank 4) and 4 gated
rank-4 experts with a timestep-selective scalar gate:

    out = x @ W.T + b + SCALE * (x @ base_down.T) @ base_up.T
          + tw * SCALE * sum_e gates[e] * (x @ expert_down[e].T) @ expert_up[e].T

Strategy (8 NeuronCores, pure data-parallel over tokens, no collectives):
  - Flatten x to [8192, 3072]; each core owns 1024 tokens.
  - Host prep folds the tiny timestep-gate softmax scalar `tw`, `gates` and
    SCALE into a single rank-21 update: coef[t,k] = x @ D  (D = [base_down;
    expert_down] as [3072,128], cols 21..127 zero),  out += coefT.T @ U with
    U rows 0..19 the scaled up-projections, row 20 the bias (coef row 20 := 1).
  - W is pre-laid-out transposed (Wt = W.T, a weight layout choice) so the
    moving matmul operand streams contiguously.
  - On device: PE-transposes x tiles once into an SBUF-resident x.T
    ([3072, 1024] per core), then accumulates the main matmul over 24 K-tiles
    into all 8 PSUM banks (one per 128-token tile) in float32r (FP22
    multiplies at full PE rate, fp32 accumulate), and folds the rank-21
    LoRA/MoE/bias update into the same PSUM accumulation group.
"""

import numpy as np

import concourse.bass as bass  # noqa: F401  (engine namespaces hang off nc)
import concourse.mybir as mybir
import concourse.tile as tile
from concourse import bacc
from concourse.bass_utils import run_bass_kernel_spmd
from concourse.masks import make_identity

F32 = mybir.dt.float32
F32R = mybir.dt.float32r

P = 128          # partitions
D = 3072         # d_model (in = out)
N_CORES = 8
T_SHARD = 1024   # tokens per core (2*4096/8)
NT = T_SHARD // P   # 8 token tiles per core
NI = D // P         # 24 contraction tiles
OB = 512            # output free-dim block (one PSUM bank, fp32 moving max)
NOB = D // OB       # 6 output blocks
KU = 128            # padded rank of the combined low-rank update
LORA_DIM = 4
SCALE = 1.0 / LORA_DIM
E = 4

_CACHED_NC = None


def _build_bass():
    """Build + compile the per-core Bass program (same on all 8 cores)."""
    nc = bacc.Bacc("TRN2", target_bir_lowering=False, debug=False,
                   num_devices=N_CORES)
    x_d = nc.dram_tensor("x", [T_SHARD, D], F32, kind="ExternalInput").ap()
    wt_d = nc.dram_tensor("wt", [D, D], F32, kind="ExternalInput").ap()
    dt_d = nc.dram_tensor("dt", [P, NI, KU], F32, kind="ExternalInput").ap()
    u_d = nc.dram_tensor("u", [KU, D], F32, kind="ExternalInput").ap()
    ones_d = nc.dram_tensor("ones", [1, T_SHARD], F32, kind="ExternalInput").ap()
    out_d = nc.dram_tensor("out", [T_SHARD, D], F32, kind="ExternalOutput").ap()

    with tile.TileContext(nc) as tc:
        with tc.tile_pool(name="consts", bufs=1) as consts, \
             tc.tile_pool(name="xtp", bufs=1) as xtp, \
             tc.tile_pool(name="stage", bufs=2) as stage_pool, \
             tc.tile_pool(name="wtp", bufs=4) as wt_pool, \
             tc.tile_pool(name="outp", bufs=4) as out_pool:

            identity = consts.tile([P, P], F32)
            make_identity(nc, identity)

            dt_sb = consts.tile([P, NI, KU], F32R)
            nc.sync.dma_start(out=dt_sb, in_=dt_d.bitcast(F32R))
            u_sb = consts.tile([KU, D], F32R)
            nc.sync.dma_start(out=u_sb, in_=u_d.bitcast(F32R))
            coefT = consts.tile([KU, T_SHARD], F32R)

            # x.T resident in SBUF: [i_part, i_tile, t]
            xt = xtp.tile([P, NI, T_SHARD], F32R)

            # ---- Phase A: load x and transpose tiles onto partitions=i ----
            # ---- Phase B: coefT[k, t] = sum_i D[i,k] * x.T[i,t] ----
            with tc.tile_pool(name="tpsum", bufs=4, space="PSUM") as tpsum, \
                 tc.tile_pool(name="cpsum", bufs=2, space="PSUM") as cpsum:
                for tt in range(NT):
                    xstage = stage_pool.tile([P, D], F32, name=f"xstage_{tt}",
                                             tag="xstage")
                    nc.sync.dma_start(out=xstage,
                                      in_=x_d[tt * P:(tt + 1) * P, :])
                    for i in range(NI):
                        pt = tpsum.tile([P, P], F32, name=f"pt_{tt}_{i}",
                                        tag="pt")
                        nc.tensor.transpose(pt, xstage[:, i * P:(i + 1) * P],
                                            identity)
                        nc.vector.tensor_copy(xt[:, i, tt * P:(tt + 1) * P], pt)

                for c in range(2):  # two 512-token chunks
                    cps = cpsum.tile([P, OB], F32, name=f"cps_{c}", tag="cps")
                    for i in range(NI):
                        nc.tensor.matmul(
                            cps,
                            lhsT=dt_sb[:, i, :],
                            rhs=xt[:, i, c * OB:(c + 1) * OB],
                            start=(i == 0), stop=(i == NI - 1),
                        )
                    nc.vector.tensor_copy(coefT[:, c * OB:(c + 1) * OB], cps)
                # constant-1 row that turns U's bias row into "+ b"
                nc.sync.dma_start(out=coefT[20:21, :],
                                  in_=ones_d.bitcast(F32R))

            # ---- Phase C: main matmul + fused rank-21 update ----
            with tc.tile_pool(name="mpsum", bufs=8, space="PSUM") as mpsum:
                for ob in range(NOB):
                    banks = [
                        mpsum.tile([P, OB], F32, name=f"bank_{ob}_{tt}",
                                   tag="bank")
                        for tt in range(NT)
                    ]
                    for i in range(NI):
                        wt_sb = wt_pool.tile([P, OB], F32R, name=f"wt_{ob}_{i}",
                                             tag="wt")
                        nc.sync.dma_start(
                            out=wt_sb,
                            in_=wt_d[i * P:(i + 1) * P,
                                     ob * OB:(ob + 1) * OB].bitcast(F32R))
                        wr = wt_sb
                        for tt in range(NT):
                            nc.tensor.matmul(
                                banks[tt],
                                lhsT=xt[:, i, tt * P:(tt + 1) * P],
                                rhs=wr,
                                start=(i == 0), stop=False,
                            )
                    for tt in range(NT):
                        nc.tensor.matmul(
                            banks[tt],
                            lhsT=coefT[:, tt * P:(tt + 1) * P],
                            rhs=u_sb[:, ob * OB:(ob + 1) * OB],
                            start=False, stop=True,
                        )
                    for tt in range(NT):
                        ot = out_pool.tile([P, OB], F32, name=f"ot_{ob}_{tt}",
                                           tag="ot")
                        nc.vector.tensor_copy(ot, banks[tt])
                        nc.sync.dma_start(
                            out=out_d[tt * P:(tt + 1) * P,
                                      ob * OB:(ob + 1) * OB],
                            in_=ot)

    nc.compile()
    return nc


def _get_nc():
    global _CACHED_NC
    if _CACHED_NC is None:
        _CACHED_NC = _build_bass()
    return _CACHED_NC


def _host_prep(x, W, b, base_down, base_up, expert_down, expert_up, gates,
               tgate_w, tgate_b, timestep):
    """Shard x; fold gate scalars into the combined low-rank matrices."""
    x = np.ascontiguousarray(np.asarray(x, np.float32).reshape(-1, D))
    W = np.asarray(W, np.float32)
    b = np.asarray(b, np.float32)
    base_down = np.asarray(base_down, np.float32)
    base_up = np.asarray(base_up, np.float32)
    expert_down = np.asarray(expert_down, np.float32)
    expert_up = np.asarray(expert_up, np.float32)
    gates = np.asarray(gates, np.float32)
    tgate_w = np.asarray(tgate_w, np.float32)
    tgate_b = np.asarray(tgate_b, np.float32)
    timestep = np.asarray(timestep, np.float32)

    # timestep-selective scalar: softmax(t @ w.T + b) . [0, 0.5, 1]
    logits = (timestep.reshape(1, 1) @ tgate_w.T + tgate_b).astype(np.float32)
    m = logits.max()
    p = np.exp(logits - m)
    p = p / p.sum()
    tw = float((p * np.array([0.0, 0.5, 1.0], np.float32)).sum())

    # Combined down matrix D[i, k]: cols 0..3 base, 4..19 experts, rest 0.
    Dc = np.zeros((D, KU), np.float32)
    Dc[:, 0:LORA_DIM] = base_down.T
    Dc[:, LORA_DIM:LORA_DIM + E * LORA_DIM] = expert_down.reshape(E * LORA_DIM, D).T
    # dt layout [P, NI, KU] so the SBUF copy is one contiguous DMA
    dt_host = np.ascontiguousarray(
        Dc.reshape(NI, P, KU).transpose(1, 0, 2))

    # Combined scaled up matrix U[k, o] (+ bias row at k=20).
    U = np.zeros((KU, D), np.float32)
    U[0:LORA_DIM] = SCALE * base_up.T
    for e in range(E):
        U[LORA_DIM + LORA_DIM * e: 2 * LORA_DIM + LORA_DIM * e] = \
            (tw * SCALE * float(gates[e])) * expert_up[e].T
    U[20] = b

    # Weight layout: contraction dim on partitions.
    Wt = np.ascontiguousarray(W.T)

    shards = [x[c * T_SHARD:(c + 1) * T_SHARD] for c in range(N_CORES)]
    return shards, Wt, dt_host, U


def _run(inputs, trace=False, trace_cores=None):
    shards, Wt, dt_host, U = _host_prep(**inputs)
    nc = _get_nc()
    in_maps = [
        {"x": shards[c], "wt": Wt, "dt": dt_host, "u": U,
         "ones": np.ones((1, T_SHARD), np.float32)}
        for c in range(N_CORES)
    ]
    res = run_bass_kernel_spmd(nc, in_maps, list(range(N_CORES)),
                               trace=trace, trace_cores=trace_cores)
    out = np.concatenate([res.results[c]["out"] for c in range(N_CORES)], axis=0)
    return out.reshape(2, 4096, D), res


def kernel(**inputs) -> np.ndarray:
    out, _ = _run(inputs)
    return out


# revision 5
# speedup vs baseline: 1.0737x; 1.0737x over previous
"""Trainium2 Bass kernel for nn_LoRAMoEModule (fused base linear + LoRA + MoE-LoRA).

Computes, for x:[B,S,D], W:[Dout,D], b:[Dout], LoRA base (rank 4) and 4 gated
rank-4 experts with a timestep-selective scalar gate:

    out = x @ W.T + b + SCALE * (x @ base_down.T) @ base_up.T
          + tw * SCALE * sum_e gates[e] * (x @ expert_down[e].T) @ expert_up[e].T

Strategy (8 NeuronCores, pure data-parallel over tokens, no collectives):
  - Flatten x to [8192, 3072]; each core owns 1024 tokens.
  - Host prep folds the tiny timestep-gate softmax scalar `tw`, `gates` and
    SCALE into a single rank-21 update: coef[t,k] = x @ D  (D = [base_down;
    expert_down] as [3072,128], cols 21..127 zero),  out += coefT.T @ U with
    U rows 0..19 the scaled up-projections, row 20 the bias (coef row 20 := 1).
  - W is pre-laid-out transposed (Wt = W.T, a weight layout choice) so the
    moving matmul operand streams contiguously.
  - On device: PE-transposes x tiles once into an SBUF-resident x.T
    ([3072, 1024] per core), then accumulates the main matmul over 24 K-tiles
    into all 8 PSUM banks (one per 128-token tile) in float32r (FP22
    multiplies at full PE rate, fp32 accumulate), and folds the rank-21
    LoRA/MoE/bias update into the same PSUM accumulation group.
"""

import numpy as np

import concourse.bass as bass  # noqa: F401  (engine namespaces hang off nc)
import concourse.mybir as mybir
import concourse.tile as tile
from concourse import bacc
from concourse.bass_utils import run_bass_kernel_spmd
from concourse.masks import make_identity

F32 = mybir.dt.float32
F32R = mybir.dt.float32r

P = 128          # partitions
D = 3072         # d_model (in = out)
N_CORES = 8
T_SHARD = 1024   # tokens per core (2*4096/8)
NT = T_SHARD // P   # 8 token tiles per core
NI = D // P         # 24 contraction tiles
OB = 512            # output free-dim block (one PSUM bank, fp32 moving max)
NOB = D // OB       # 6 output blocks
KU = 128            # padded rank of the combined low-rank update
LORA_DIM = 4
SCALE = 1.0 / LORA_DIM
E = 4

_CACHED_NC = None


def _build_bass():
    """Build + compile the per-core Bass program (same on all 8 cores)."""
    nc = bacc.Bacc("TRN2", target_bir_lowering=False, debug=False,
                   num_devices=N_CORES)
    x_d = nc.dram_tensor("x", [T_SHARD, D], F32, kind="ExternalInput").ap()
    wt_d = nc.dram_tensor("wt", [D, D], F32, kind="ExternalInput").ap()
    dt_d = nc.dram_tensor("dt", [P, NI, KU], F32, kind="ExternalInput").ap()
    u_d = nc.dram_tensor("u", [KU, D], F32, kind="ExternalInput").ap()
    ones_d = nc.dram_tensor("ones", [1, T_SHARD], F32, kind="ExternalInput").ap()
    out_d = nc.dram_tensor("out", [T_SHARD, D], F32, kind="ExternalOutput").ap()

    with tile.TileContext(nc) as tc:
        with tc.tile_pool(name="consts", bufs=1) as consts, \
             tc.tile_pool(name="xtp", bufs=1) as xtp, \
             tc.tile_pool(name="stage", bufs=2) as stage_pool, \
             tc.tile_pool(name="wtp", bufs=16) as wt_pool, \
             tc.tile_pool(name="outp", bufs=4) as out_pool:

            identity_f = consts.tile([P, P], F32)
            make_identity(nc, identity_f)
            identity = consts.tile([P, P], F32R)
            nc.vector.tensor_copy(identity, identity_f)

            dt_sb = consts.tile([P, NI, KU], F32R)
            u_sb = consts.tile([KU, D], F32R)
            coefT = consts.tile([KU, T_SHARD], F32R)

            # x.T resident in SBUF: [i_part, i_tile, t]
            xt = xtp.tile([P, NI, T_SHARD], F32R)

            # ---- Phase A: load x and transpose tiles onto partitions=i ----
            # ---- Phase B: coefT[k, t] = sum_i D[i,k] * x.T[i,t] ----
            with tc.tile_pool(name="tpsum", bufs=4, space="PSUM") as tpsum, \
                 tc.tile_pool(name="cpsum", bufs=2, space="PSUM") as cpsum:
                for tt in range(NT):
                    xstage = stage_pool.tile([P, D], F32R, name=f"xstage_{tt}",
                                             tag="xstage", bufs=3)
                    nc.gpsimd.dma_start(out=xstage,
                                        in_=x_d[tt * P:(tt + 1) * P,
                                                :].bitcast(F32R))
                    for i in range(NI):
                        pt = tpsum.tile([P, P], F32R, name=f"pt_{tt}_{i}",
                                        tag="pt")
                        nc.tensor.transpose(pt, xstage[:, i * P:(i + 1) * P],
                                            identity)
                        nc.vector.tensor_copy(xt[:, i, tt * P:(tt + 1) * P], pt)
                # small consts after the x stream (SWDGE is FIFO; consumers
                # of these run late)
                nc.gpsimd.dma_start(out=dt_sb, in_=dt_d.bitcast(F32R))
                nc.gpsimd.dma_start(out=u_sb, in_=u_d.bitcast(F32R))

                for c in range(2):  # two 512-token chunks
                    cps = cpsum.tile([P, OB], F32, name=f"cps_{c}", tag="cps")
                    for i in range(NI):
                        nc.tensor.matmul(
                            cps,
                            lhsT=dt_sb[:, i, :],
                            rhs=xt[:, i, c * OB:(c + 1) * OB],
                            start=(i == 0), stop=(i == NI - 1),
                        )
                    nc.vector.tensor_copy(coefT[:, c * OB:(c + 1) * OB], cps)
                # constant-1 row that turns U's bias row into "+ b"
                nc.gpsimd.dma_start(out=coefT[20:21, :],
                                    in_=ones_d.bitcast(F32R))

            # ---- Phase C: main matmul + fused rank-21 update ----
            with tc.tile_pool(name="mpsum", bufs=8, space="PSUM") as mpsum:
                for ob in range(NOB):
                    banks = [
                        mpsum.tile([P, OB], F32, name=f"bank_{ob}_{tt}",
                                   tag="bank")
                        for tt in range(NT)
                    ]
                    for i in range(NI):
                        wt_sb = wt_pool.tile([P, OB], F32R, name=f"wt_{ob}_{i}",
                                             tag="wt")
                        nc.sync.dma_start(
                            out=wt_sb,
                            in_=wt_d[i * P:(i + 1) * P,
                                     ob * OB:(ob + 1) * OB].bitcast(F32R))
                        wr = wt_sb
                        last = (i == NI - 1)
                        for tt in range(NT):
                            nc.tensor.matmul(
                                banks[tt],
                                lhsT=xt[:, i, tt * P:(tt + 1) * P],
                                rhs=wr,
                                start=(i == 0), stop=False,
                            )
                            if last:
                                # fused LoRA/MoE/bias update closes this
                                # bank's accumulation group immediately so
                                # its evacuation overlaps the remaining MMs
                                nc.tensor.matmul(
                                    banks[tt],
                                    lhsT=coefT[:, tt * P:(tt + 1) * P],
                                    rhs=u_sb[:, ob * OB:(ob + 1) * OB],
                                    start=False, stop=True,
                                )
                                ot = out_pool.tile([P, OB], F32,
                                                   name=f"ot_{ob}_{tt}",
                                                   tag="ot")
                                nc.vector.tensor_copy(ot, banks[tt])
                                nc.sync.dma_start(
                                    out=out_d[tt * P:(tt + 1) * P,
                                              ob * OB:(ob + 1) * OB],
                                    in_=ot)

    nc.compile()
    return nc


def _get_nc():
    global _CACHED_NC
    if _CACHED_NC is None:
        _CACHED_NC = _build_bass()
    return _CACHED_NC


def _host_prep(x, W, b, base_down, base_up, expert_down, expert_up, gates,
               tgate_w, tgate_b, timestep):
    """Shard x; fold gate scalars into the combined low-rank matrices."""
    x = np.ascontiguousarray(np.asarray(x, np.float32).reshape(-1, D))
    W = np.asarray(W, np.float32)
    b = np.asarray(b, np.float32)
    base_down = np.asarray(base_down, np.float32)
    base_up = np.asarray(base_up, np.float32)
    expert_down = np.asarray(expert_down, np.float32)
    expert_up = np.asarray(expert_up, np.float32)
    gates = np.asarray(gates, np.float32)
    tgate_w = np.asarray(tgate_w, np.float32)
    tgate_b = np.asarray(tgate_b, np.float32)
    timestep = np.asarray(timestep, np.float32)

    # timestep-selective scalar: softmax(t @ w.T + b) . [0, 0.5, 1]
    logits = (timestep.reshape(1, 1) @ tgate_w.T + tgate_b).astype(np.float32)
    m = logits.max()
    p = np.exp(logits - m)
    p = p / p.sum()
    tw = float((p * np.array([0.0, 0.5, 1.0], np.float32)).sum())

    # Combined down matrix D[i, k]: cols 0..3 base, 4..19 experts, rest 0.
    Dc = np.zeros((D, KU), np.float32)
    Dc[:, 0:LORA_DIM] = base_down.T
    Dc[:, LORA_DIM:LORA_DIM + E * LORA_DIM] = expert_down.reshape(E * LORA_DIM, D).T
    # dt layout [P, NI, KU] so the SBUF copy is one contiguous DMA
    dt_host = np.ascontiguousarray(
        Dc.reshape(NI, P, KU).transpose(1, 0, 2))

    # Combined scaled up matrix U[k, o] (+ bias row at k=20).
    U = np.zeros((KU, D), np.float32)
    U[0:LORA_DIM] = SCALE * base_up.T
    for e in range(E):
        U[LORA_DIM + LORA_DIM * e: 2 * LORA_DIM + LORA_DIM * e] = \
            (tw * SCALE * float(gates[e])) * expert_up[e].T
    U[20] = b

    # Weight layout: contraction dim on partitions.
    Wt = np.ascontiguousarray(W.T)

    shards = [x[c * T_SHARD:(c + 1) * T_SHARD] for c in range(N_CORES)]
    return shards, Wt, dt_host, U


def _run(inputs, trace=False, trace_cores=None):
    shards, Wt, dt_host, U = _host_prep(**inputs)
    nc = _get_nc()
    in_maps = [
        {"x": shards[c], "wt": Wt, "dt": dt_host, "u": U,
         "ones": np.ones((1, T_SHARD), np.float32)}
        for c in range(N_CORES)
    ]
    res = run_bass_kernel_spmd(nc, in_maps, list(range(N_CORES)),
                               trace=trace, trace_cores=trace_cores)
    out = np.concatenate([res.results[c]["out"] for c in range(N_CORES)], axis=0)
    return out.reshape(2, 4096, D), res


def kernel(**inputs) -> np.ndarray:
    out, _ = _run(inputs)
    return out
